# revision 27
# baseline (speedup 1.0000x reference)
"""Multi-head attention (B=4, S=2048, C=1024, H=16) on 8 TRN2 NeuronCores.

Sharding: data-parallel over batch (4) x query-row split (2). Core c handles
batch c//2, query rows [(c%2)*1024, (c%2)*1024+1024). Each core computes the
QKV projection for its batch (K/V over the full sequence, Q over its own rows)
with float32r matmuls, spills Q^T/K^T/V to DRAM scratch, then runs per-head
attention with transposed scores ([key, query] layout) so the softmaxed
probabilities feed the P.V matmul directly as the moving operand. A ones
column appended to V accumulates the softmax denominator in the same PSUM
tile. The out-projection uses O^T as the stationary operand so the result is
produced in natural [row, channel] layout. No collectives.
"""

from contextlib import ExitStack

import numpy as np

import concourse.bass as bass
import concourse.mybir as mybir
import concourse.tile as tile
from concourse import bacc
from concourse.bass_utils import run_bass_kernel_spmd
from concourse.masks import make_identity

F32 = mybir.dt.float32
F32R = mybir.dt.float32r
AF = mybir.ActivationFunctionType

B, S, C, H, DH = 4, 2048, 1024, 16, 64
NCORES = 8
SCALE = DH ** -0.5  # 0.125
CT = C // 128  # 8 channel tiles
ST = S // 128  # 16 seq tiles
MYROWS = S // 2  # 1024 query rows per core


def _transpose_group(nc, tp_pool, ident, src, cts, dst_ap):
    """PE-transpose 4 [128,128] blocks of src (channel tiles cts) and copy the
    [128, 512] group to dst_ap (a strided AP covering the 4 destinations)."""
    tp = tp_pool.tile([128, 512], F32)
    for k, ct in enumerate(cts):
        nc.tensor.transpose(tp[:, k * 128:(k + 1) * 128],
                            src[:, ct * 128:(ct + 1) * 128], ident)
    nc.vector.tensor_copy(dst_ap, tp[:])


def build():
    nc = bacc.Bacc("TRN2", target_bir_lowering=False, debug=False,
                   num_devices=NCORES)

    # host-prepared layouts (pure data movement on the host):
    #   xT[c, s] = x[s, c]
    #   wqk[wt, p, ct*128+f] = W_qkv[wt*128+f, ct*128+p]   (Q/K strips)
    #   wv[vch, p, ct*512+f] = W_qkv[2C+vch*512+f, ct*128+p]
    #   wo[et, p, ct*512+e] = W_out[et*512+e, ct*128+p]
    #   bq2d[p, wt] = b_qkv[wt*128+p]
    xT_in = nc.dram_tensor("xT", [C, S], F32R, kind="ExternalInput").ap()
    wqk_in = nc.dram_tensor("wqk", [16, 128, CT * 128], F32R,
                            kind="ExternalInput").ap()
    wv_in = nc.dram_tensor("wv", [2, 128, CT * 512], F32R,
                           kind="ExternalInput").ap()
    wo_in = nc.dram_tensor("wo", [2, 128, CT * 512], F32R,
                           kind="ExternalInput").ap()
    bq2d = nc.dram_tensor("bq2d", [128, 3 * C // 128], F32,
                          kind="ExternalInput").ap()
    b_out = nc.dram_tensor("b_out", [C], F32, kind="ExternalInput").ap()
    out = nc.dram_tensor("out", [MYROWS, C], F32, kind="ExternalOutput").ap()

    # DRAM scratch for the projected tensors (feature-major Q^T/K^T, natural V)
    qT_d = nc.dram_tensor("qT_d", [C, MYROWS], F32R).ap()
    kT_d = nc.dram_tensor("kT_d", [C, S], F32R).ap()
    v_d = nc.dram_tensor("v_d", [S, C], F32R).ap()

    with tile.TileContext(nc) as tc, ExitStack() as ctx:
        const = ctx.enter_context(tc.tile_pool(name="const", bufs=1))
        ident = const.tile([128, 128], F32)
        make_identity(nc, ident[:])  # used for the ones-column writes

        b_sb = const.tile([128, 3 * C // 128], F32)  # b_sb[p, wt] = b_qkv[wt*128+p]
        nc.sync.dma_start(b_sb[:], bq2d)
        bo_sb = const.tile([1, C], F32)
        nc.sync.dma_start(bo_sb[:], b_out[None, :])
        bo_bc = const.tile([128, C], F32)
        nc.gpsimd.partition_broadcast(bo_bc[:], bo_sb[0:1, :])

        persist = ctx.enter_context(tc.tile_pool(name="persist", bufs=1))
        xT = persist.tile([128, CT * S], F32R)  # xT[p, ct*S + s] = x[s, ct*128+p]

        # ---------------- Phase A: QKV projection ----------------
        with ExitStack() as actx:
            wstrip = actx.enter_context(tc.tile_pool(name="wstrip", bufs=3))
            vw_pool = actx.enter_context(tc.tile_pool(name="vw", bufs=2))
            stage = actx.enter_context(tc.tile_pool(name="stage", bufs=4))
            acc_ps = actx.enter_context(
                tc.tile_pool(name="acc_ps", bufs=3, space="PSUM"))

            # x^T resident: 8 c-tiles of [128, S]
            for ct in range(CT):
                nc.sync.dma_start(xT[:, ct * S:(ct + 1) * S],
                                  xT_in[ct * 128:(ct + 1) * 128, :])

            # Q^T / K^T: per 128-feature strip, stream W^T column slices
            for wt in range(16):
                # ws[p, ct, f] = wT[ct*128+p, wt*128+f]
                ws = wstrip.tile([128, CT * 128], F32R)
                nc.sync.dma_start(ws[:], wqk_in[wt])
                if wt < 8:
                    nsch, dest, drow = 2, qT_d, wt
                else:
                    nsch, dest, drow = 4, kT_d, wt - 8
                for sch in range(nsch):
                    acc = acc_ps.tile([128, 512], F32)
                    for ct in range(CT):
                        nc.tensor.matmul(
                            acc[:],
                            ws[:, ct * 128:(ct + 1) * 128],
                            xT[:, ct * S + sch * 512: ct * S + sch * 512 + 512],
                            start=(ct == 0), stop=(ct == CT - 1))
                    stg = stage.tile([128, 512], F32R)
                    nc.vector.tensor_scalar_add(stg[:], acc[:],
                                                b_sb[:, wt:wt + 1])
                    nc.sync.dma_start(
                        dest[drow * 128:(drow + 1) * 128,
                             sch * 512:(sch + 1) * 512], stg[:])

            # V natural: rhs = W_v^T chunks [c_part, ct, 512 feats]
            for vch in range(2):
                vw = vw_pool.tile([128, CT * 512], F32R)
                nc.sync.dma_start(vw[:], wv_in[vch])
                for st in range(ST):
                    acc = acc_ps.tile([128, 512], F32)
                    for ct in range(CT):
                        nc.tensor.matmul(
                            acc[:],
                            xT[:, ct * S + st * 128: ct * S + (st + 1) * 128],
                            vw[:, ct * 512:(ct + 1) * 512],
                            start=(ct == 0), stop=(ct == CT - 1))
                    stg = stage.tile([128, 512], F32R)
                    nc.vector.tensor_copy(stg[:], acc[:])
                    nc.sync.dma_start(
                        v_d[st * 128:(st + 1) * 128,
                            vch * 512:(vch + 1) * 512], stg[:])

        # ---------------- Phase B: attention ----------------
        OT = persist.tile([128, CT * MYROWS], F32R)  # OT[p, ct*1024 + i]
        with ExitStack() as bctx:
            kp = bctx.enter_context(tc.tile_pool(name="kp", bufs=2))
            vp = bctx.enter_context(tc.tile_pool(name="vp", bufs=1))
            qp = bctx.enter_context(tc.tile_pool(name="qp", bufs=2))
            pp = bctx.enter_context(tc.tile_pool(name="pp", bufs=2))
            smalls = bctx.enter_context(tc.tile_pool(name="smalls", bufs=3))
            sc_ps = bctx.enter_context(
                tc.tile_pool(name="sc_ps", bufs=1, space="PSUM"))
            pv_ps = bctx.enter_context(
                tc.tile_pool(name="pv_ps", bufs=1, space="PSUM"))

            for hp in range(H // 2):  # head pairs: A = rows 0-63, B = 64-127
                kt = kp.tile([128, S], F32R)
                nc.sync.dma_start(kt[:], kT_d[hp * 128:(hp + 1) * 128, :])
                # pair-wide V tile, padded to 128 stationary columns per
                # (j-tile, head): [V_A(64)|1|0*63|V_B(64)|1|0*63] per j-tile.
                # Full-width weights keep the fp32r fast-weight-load path;
                # the ones column accumulates the softmax denominator at out
                # row 64. One contiguous-chunk 1 MB DMA loads both heads.
                vt = vp.tile([128, ST * 256], F32R)
                vt4 = vt[:].rearrange("p (t g f) -> p t g f", g=2, f=128)
                nc.vector.tensor_scalar(
                    vt4[:, :, :, DH:DH + 1],
                    ident[:, 0:2 * ST].rearrange("p (t g) -> p t g", g=2),
                    0.0, 1.0, mybir.AluOpType.mult, mybir.AluOpType.add)
                for g in range(2):
                    nc.vector.tensor_scalar(
                        vt4[:, :, g:g + 1, DH + 1:128],
                        bo_bc[:, 0:ST * (127 - DH)].rearrange(
                            "p (t g f) -> p t g f", g=1, f=127 - DH),
                        0.0, 0.0, mybir.AluOpType.mult, mybir.AluOpType.mult)
                for g in range(2):
                    nc.sync.dma_start(
                        vt4[:, :, g:g + 1, 0:DH],
                        v_d.rearrange("(t p) (g f) -> p t g f", p=128, f=64)[
                            :, :, 2 * hp + g:2 * hp + g + 1, :])
                for ich in range(2):
                    qt = qp.tile([128, 512], F32R)
                    nc.sync.dma_start(
                        qt[:], qT_d[hp * 128:(hp + 1) * 128,
                                    ich * 512:(ich + 1) * 512])
                    pvs = [pv_ps.tile([128, 512], F32, tag=f"pv{half}",
                                      name=f"pv{half}")
                           for half in range(2)]
                    def emit_pv(pg_prev, js_prev):
                        for half in range(2):
                            for idx, j in enumerate(js_prev):
                                nc.tensor.matmul(
                                    pvs[half][:],
                                    vt[:, j * 256 + half * 128:
                                       j * 256 + half * 128 + 128],
                                    pg_prev[:, (half * 3 + idx) * 512:
                                            (half * 3 + idx + 1) * 512],
                                    start=(j == 0), stop=(j == 15))

                    jb = 0
                    prev = None
                    for blk in (2, 3, 3, 3, 3, 1, 1):
                        js = list(range(jb, jb + blk))
                        jb += blk
                        # one PSUM tile for both heads: A cols [0,1536),
                        # B cols [1536,3072) -> a single exp instruction
                        sc = sc_ps.tile([128, 6 * 512], F32)
                        for idx, j in enumerate(js):
                            # row-packed pair: head A on PE rows 0-63,
                            # head B on rows 64-127, concurrent
                            for half in range(2):
                                p0 = half * 64
                                nc.tensor.matmul(
                                    sc[:, (half * 3 + idx) * 512:
                                       (half * 3 + idx + 1) * 512],
                                    kt[p0:p0 + 64, j * 128:(j + 1) * 128],
                                    qt[p0:p0 + 64, :],
                                    start=True, stop=True)
                        pg = pp.tile([128, 6 * 512], F32R)
                        if blk == 3:
                            nc.scalar.activation(pg[:], sc[:], AF.Exp,
                                                 scale=SCALE)
                        else:
                            for half in range(2):
                                nc.scalar.activation(
                                    pg[:, half * 1536:half * 1536 + blk * 512],
                                    sc[:, half * 1536:half * 1536 + blk * 512],
                                    AF.Exp, scale=SCALE)
                        # software pipeline: PV of the previous block runs
                        # while ACT computes this block's exp
                        if prev is not None:
                            emit_pv(*prev)
                        prev = (pg, js)
                    emit_pv(*prev)
                    for half in range(2):
                        pv = pvs[half]
                        # copy [out|denom] rows out of PSUM immediately so the
                        # accumulator bank frees for the next iteration
                        uv = smalls.tile([65, 512], F32)
                        nc.vector.tensor_copy(uv[:], pv[0:65, :])
                        rec = smalls.tile([1, 512], F32)
                        nc.vector.reciprocal(rec[:], uv[64:65, :])
                        rb = smalls.tile([64, 512], F32)
                        nc.gpsimd.partition_broadcast(rb[:], rec[0:1, :])
                        o2 = smalls.tile([64, 512], F32)
                        nc.vector.tensor_mul(o2[:], uv[0:64, :], rb[:])
                        oslice = OT[half * 64:half * 64 + 64,
                                    hp * MYROWS + ich * 512:
                                    hp * MYROWS + (ich + 1) * 512]
                        nc.vector.tensor_scalar_add(
                            oslice, o2[:],
                            b_sb[half * 64:half * 64 + 64, 16 + hp:17 + hp])

        # ---------------- Phase C: out projection ----------------
        with ExitStack() as cctx:
            woT_pool = cctx.enter_context(tc.tile_pool(name="woT", bufs=2))
            yt_pool = cctx.enter_context(tc.tile_pool(name="yt", bufs=3))
            y_ps = cctx.enter_context(
                tc.tile_pool(name="y_ps", bufs=2, space="PSUM"))

            for et in range(2):
                woT = woT_pool.tile([128, CT * 512], F32R)  # [c_p, ct, 512 e]
                nc.sync.dma_start(woT[:], wo_in[et])
                for it in range(8):
                    y = y_ps.tile([128, 512], F32)
                    for ct in range(CT):
                        nc.tensor.matmul(
                            y[:],
                            OT[:, ct * MYROWS + it * 128: ct * MYROWS + (it + 1) * 128],
                            woT[:, ct * 512:(ct + 1) * 512],
                            start=(ct == 0), stop=(ct == CT - 1))
                    yt = yt_pool.tile([128, 512], F32)
                    nc.vector.tensor_add(yt[:], y[:], bo_bc[:, et * 512:(et + 1) * 512])
                    nc.sync.dma_start(
                        out[it * 128:(it + 1) * 128, et * 512:(et + 1) * 512], yt[:])

    nc.compile()
    return nc


_cache = {}


def _get_nc():
    if "nc" not in _cache:
        _cache["nc"] = build()
    return _cache["nc"]


def kernel(x_q, W_qkv, b_qkv, W_out, b_out):
    """Core c of 8 handles batch c//2, query rows [(c%2)*1024, +1024).

    The per-core x slice is ROLLED by the core's query-row offset so every
    core's own query rows sit at rows [0, MYROWS) of its slice. Attention is
    permutation-invariant over keys, so the rolled K/V ordering does not
    change the output.
    """
    x_q = np.ascontiguousarray(x_q, dtype=np.float32)
    W_qkv = np.ascontiguousarray(W_qkv, dtype=np.float32)
    b_qkv = np.ascontiguousarray(b_qkv, dtype=np.float32)
    W_out = np.ascontiguousarray(W_out, dtype=np.float32)
    b_out = np.ascontiguousarray(b_out, dtype=np.float32)

    nc = _get_nc()
    in_maps = build_in_maps(x_q, W_qkv, b_qkv, W_out, b_out)
    res = run_bass_kernel_spmd(nc, in_maps, list(range(NCORES)))
    out = np.empty((B, S, C), dtype=np.float32)
    for c in range(NCORES):
        b, half = c // 2, c % 2
        out[b, half * MYROWS:(half + 1) * MYROWS] = res.results[c]["out"]
    return out


def build_in_maps(x_q, W_qkv, b_qkv, W_out, b_out):
    x_q = np.ascontiguousarray(x_q, dtype=np.float32)
    W_qkv = np.asarray(W_qkv, dtype=np.float32)
    b_qkv = np.ascontiguousarray(b_qkv, dtype=np.float32)
    W_out = np.asarray(W_out, dtype=np.float32)
    b_out = np.ascontiguousarray(b_out, dtype=np.float32)
    # wqk[wt, p, ct*128+f] = W_qkv[wt*128+f, ct*128+p]
    w4 = W_qkv.reshape(24, 128, CT, 128)            # [wt, f, ct, p]
    wqk = np.ascontiguousarray(w4[:16].transpose(0, 3, 2, 1).reshape(
        16, 128, CT * 128))
    # wv[vch, p, ct*512+f] = W_qkv[2C+vch*512+f, ct*128+p]
    wv5 = W_qkv[2 * C:].reshape(2, 512, CT, 128)    # [vch, f, ct, p]
    wv = np.ascontiguousarray(wv5.transpose(0, 3, 2, 1).reshape(
        2, 128, CT * 512))
    # wo[et, p, ct*512+e] = W_out[et*512+e, ct*128+p]
    wo5 = W_out.reshape(2, 512, CT, 128)            # [et, e, ct, p]
    wo = np.ascontiguousarray(wo5.transpose(0, 3, 2, 1).reshape(
        2, 128, CT * 512))
    bq2d = np.ascontiguousarray(b_qkv.reshape(24, 128).T)
    in_maps = []
    for c in range(NCORES):
        b, half = c // 2, c % 2
        xb = x_q[b]
        if half:
            xb = np.roll(xb, -MYROWS, axis=0)
        in_maps.append({
            "xT": np.ascontiguousarray(xb.T),
            "wqk": wqk,
            "wv": wv,
            "wo": wo,
            "bq2d": bq2d,
            "b_out": b_out,
        })
    return in_maps


if __name__ == "__main__":
    # smoke test with random inputs
    rng = np.random.default_rng(0)
    x_q = rng.standard_normal((B, S, C), dtype=np.float32)
    s = 1.0 / np.sqrt(C)
    W_qkv = rng.uniform(-s, s, (3 * C, C)).astype(np.float32)
    b_qkv = rng.uniform(-s, s, 3 * C).astype(np.float32)
    W_out = rng.uniform(-s, s, (C, C)).astype(np.float32)
    b_out = rng.uniform(-s, s, C).astype(np.float32)
    got = kernel(x_q=x_q, W_qkv=W_qkv, b_qkv=b_qkv, W_out=W_out, b_out=b_out)
    print("smoke ok", got.shape, float(np.abs(got).max()))


# revision 28
# speedup vs baseline: 1.0882x; 1.0882x over previous
"""Multi-head attention (B=4, S=2048, C=1024, H=16) on 8 TRN2 NeuronCores.

Sharding: data-parallel over batch (4) x query-row split (2). Core c handles
batch c//2, query rows [(c%2)*1024, (c%2)*1024+1024). Each core computes the
QKV projection for its batch (K/V over the full sequence, Q over its own rows)
with float32r matmuls, spills Q^T/K^T/V to DRAM scratch, then runs per-head
attention with transposed scores ([key, query] layout) so the softmaxed
probabilities feed the P.V matmul directly as the moving operand. A ones
column appended to V accumulates the softmax denominator in the same PSUM
tile. The out-projection uses O^T as the stationary operand so the result is
produced in natural [row, channel] layout. No collectives.
"""

from contextlib import ExitStack

import numpy as np

import concourse.bass as bass
import concourse.mybir as mybir
import concourse.tile as tile
from concourse import bacc
from concourse.bass_utils import run_bass_kernel_spmd
from concourse.masks import make_identity

F32 = mybir.dt.float32
F32R = mybir.dt.float32r
AF = mybir.ActivationFunctionType

B, S, C, H, DH = 4, 2048, 1024, 16, 64
NCORES = 8
SCALE = DH ** -0.5  # 0.125
CT = C // 128  # 8 channel tiles
ST = S // 128  # 16 seq tiles
MYROWS = S // 2  # 1024 query rows per core


def _transpose_group(nc, tp_pool, ident, src, cts, dst_ap):
    """PE-transpose 4 [128,128] blocks of src (channel tiles cts) and copy the
    [128, 512] group to dst_ap (a strided AP covering the 4 destinations)."""
    tp = tp_pool.tile([128, 512], F32)
    for k, ct in enumerate(cts):
        nc.tensor.transpose(tp[:, k * 128:(k + 1) * 128],
                            src[:, ct * 128:(ct + 1) * 128], ident)
    nc.vector.tensor_copy(dst_ap, tp[:])


def build():
    nc = bacc.Bacc("TRN2", target_bir_lowering=False, debug=False,
                   num_devices=NCORES)

    # host-prepared layouts (pure data movement on the host):
    #   xT[c, s] = x[s, c]
    #   wqk[wt, p, ct*128+f] = W_qkv[wt*128+f, ct*128+p]   (Q/K strips)
    #   wv[vch, p, ct*512+f] = W_qkv[2C+vch*512+f, ct*128+p]
    #   wo[et, p, ct*512+e] = W_out[et*512+e, ct*128+p]
    #   bq2d[p, wt] = b_qkv[wt*128+p]
    xT_in = nc.dram_tensor("xT", [C, S], F32R, kind="ExternalInput").ap()
    wqk_in = nc.dram_tensor("wqk", [16, 128, CT * 128], F32R,
                            kind="ExternalInput").ap()
    wv_in = nc.dram_tensor("wv", [2, 128, CT * 512], F32R,
                           kind="ExternalInput").ap()
    wo_in = nc.dram_tensor("wo", [2, 128, CT * 512], F32R,
                           kind="ExternalInput").ap()
    bq2d = nc.dram_tensor("bq2d", [128, 3 * C // 128], F32,
                          kind="ExternalInput").ap()
    b_out = nc.dram_tensor("b_out", [C], F32, kind="ExternalInput").ap()
    out = nc.dram_tensor("out", [MYROWS, C], F32, kind="ExternalOutput").ap()

    # DRAM scratch for the projected tensors (feature-major Q^T/K^T, natural V)
    qT_d = nc.dram_tensor("qT_d", [C, MYROWS], F32R).ap()
    kT_d = nc.dram_tensor("kT_d", [C, S], F32R).ap()
    v_d = nc.dram_tensor("v_d", [S, C], F32R).ap()

    with tile.TileContext(nc) as tc, ExitStack() as ctx:
        const = ctx.enter_context(tc.tile_pool(name="const", bufs=1))
        ident = const.tile([128, 128], F32)
        make_identity(nc, ident[:])  # used for the ones-column writes

        b_sb = const.tile([128, 3 * C // 128], F32)  # b_sb[p, wt] = b_qkv[wt*128+p]
        nc.sync.dma_start(b_sb[:], bq2d)
        bo_sb = const.tile([1, C], F32)
        nc.sync.dma_start(bo_sb[:], b_out[None, :])
        bo_bc = const.tile([128, C], F32)
        nc.gpsimd.partition_broadcast(bo_bc[:], bo_sb[0:1, :])

        persist = ctx.enter_context(tc.tile_pool(name="persist", bufs=1))
        xT = persist.tile([128, CT * S], F32R)  # xT[p, ct*S + s] = x[s, ct*128+p]

        # ---------------- Phase A: QKV projection ----------------
        with ExitStack() as actx:
            wstrip = actx.enter_context(tc.tile_pool(name="wstrip", bufs=3))
            vw_pool = actx.enter_context(tc.tile_pool(name="vw", bufs=2))
            stage = actx.enter_context(tc.tile_pool(name="stage", bufs=4))
            acc_ps = actx.enter_context(
                tc.tile_pool(name="acc_ps", bufs=3, space="PSUM"))

            # x^T resident: 8 c-tiles of [128, S]
            for ct in range(CT):
                nc.sync.dma_start(xT[:, ct * S:(ct + 1) * S],
                                  xT_in[ct * 128:(ct + 1) * 128, :])

            # Q^T / K^T: per 128-feature strip, stream W^T column slices
            for wt in range(16):
                # ws[p, ct, f] = wT[ct*128+p, wt*128+f]
                ws = wstrip.tile([128, CT * 128], F32R)
                nc.sync.dma_start(ws[:], wqk_in[wt])
                if wt < 8:
                    nsch, dest, drow = 2, qT_d, wt
                else:
                    nsch, dest, drow = 4, kT_d, wt - 8
                for sch in range(nsch):
                    acc = acc_ps.tile([128, 512], F32)
                    for ct in range(CT):
                        nc.tensor.matmul(
                            acc[:],
                            ws[:, ct * 128:(ct + 1) * 128],
                            xT[:, ct * S + sch * 512: ct * S + sch * 512 + 512],
                            start=(ct == 0), stop=(ct == CT - 1))
                    stg = stage.tile([128, 512], F32R)
                    nc.vector.tensor_scalar_add(stg[:], acc[:],
                                                b_sb[:, wt:wt + 1])
                    nc.sync.dma_start(
                        dest[drow * 128:(drow + 1) * 128,
                             sch * 512:(sch + 1) * 512], stg[:])

            # V natural: rhs = W_v^T chunks [c_part, ct, 512 feats]
            for vch in range(2):
                vw = vw_pool.tile([128, CT * 512], F32R)
                nc.sync.dma_start(vw[:], wv_in[vch])
                for st in range(ST):
                    acc = acc_ps.tile([128, 512], F32)
                    for ct in range(CT):
                        nc.tensor.matmul(
                            acc[:],
                            xT[:, ct * S + st * 128: ct * S + (st + 1) * 128],
                            vw[:, ct * 512:(ct + 1) * 512],
                            start=(ct == 0), stop=(ct == CT - 1))
                    stg = stage.tile([128, 512], F32R)
                    nc.vector.tensor_copy(stg[:], acc[:])
                    nc.sync.dma_start(
                        v_d[st * 128:(st + 1) * 128,
                            vch * 512:(vch + 1) * 512], stg[:])

        # ---------------- Phase B: attention ----------------
        OT = persist.tile([128, CT * MYROWS], F32R)  # OT[p, ct*1024 + i]
        with ExitStack() as bctx:
            kp = bctx.enter_context(tc.tile_pool(name="kp", bufs=2))
            vp = bctx.enter_context(tc.tile_pool(name="vp", bufs=2))
            qp = bctx.enter_context(tc.tile_pool(name="qp", bufs=2))
            pp = bctx.enter_context(tc.tile_pool(name="pp", bufs=2))
            smalls = bctx.enter_context(tc.tile_pool(name="smalls", bufs=3))
            sc_ps = bctx.enter_context(
                tc.tile_pool(name="sc_ps", bufs=1, space="PSUM"))
            pv_ps = bctx.enter_context(
                tc.tile_pool(name="pv_ps", bufs=1, space="PSUM"))

            for hp in range(H // 2):  # head pairs: A = rows 0-63, B = 64-127
                kt = kp.tile([128, S], F32R)
                nc.sync.dma_start(kt[:], kT_d[hp * 128:(hp + 1) * 128, :])
                # pair-wide V tile, padded to 128 stationary columns per
                # (j-tile, head): [V_A(64)|1|0*63|V_B(64)|1|0*63] per j-tile.
                # Full-width weights keep the fp32r fast-weight-load path;
                # the ones column accumulates the softmax denominator at out
                # row 64. One contiguous-chunk 1 MB DMA loads both heads.
                vt = vp.tile([128, ST * 256], F32R)
                vt4 = vt[:].rearrange("p (t g f) -> p t g f", g=2, f=128)
                nc.vector.tensor_scalar(
                    vt4[:, :, :, DH:DH + 1],
                    ident[:, 0:2 * ST].rearrange("p (t g) -> p t g", g=2),
                    0.0, 1.0, mybir.AluOpType.mult, mybir.AluOpType.add)
                for g in range(2):
                    nc.vector.tensor_scalar(
                        vt4[:, :, g:g + 1, DH + 1:128],
                        bo_bc[:, 0:ST * (127 - DH)].rearrange(
                            "p (t g f) -> p t g f", g=1, f=127 - DH),
                        0.0, 0.0, mybir.AluOpType.mult, mybir.AluOpType.mult)
                for g in range(2):
                    nc.sync.dma_start(
                        vt4[:, :, g:g + 1, 0:DH],
                        v_d.rearrange("(t p) (g f) -> p t g f", p=128, f=64)[
                            :, :, 2 * hp + g:2 * hp + g + 1, :])
                for ich in range(2):
                    qt = qp.tile([128, 512], F32R)
                    nc.sync.dma_start(
                        qt[:], qT_d[hp * 128:(hp + 1) * 128,
                                    ich * 512:(ich + 1) * 512])
                    pvs = [pv_ps.tile([128, 512], F32, tag=f"pv{half}",
                                      name=f"pv{half}")
                           for half in range(2)]
                    def emit_pv(pg_prev, js_prev):
                        for half in range(2):
                            for idx, j in enumerate(js_prev):
                                nc.tensor.matmul(
                                    pvs[half][:],
                                    vt[:, j * 256 + half * 128:
                                       j * 256 + half * 128 + 128],
                                    pg_prev[:, (half * 3 + idx) * 512:
                                            (half * 3 + idx + 1) * 512],
                                    start=(j == 0), stop=(j == 15))

                    jb = 0
                    prev = None
                    for blk in (3, 3, 3, 3, 2, 2):
                        js = list(range(jb, jb + blk))
                        jb += blk
                        # one PSUM tile for both heads: A cols [0,1536),
                        # B cols [1536,3072) -> a single exp instruction
                        sc = sc_ps.tile([128, 6 * 512], F32)
                        for idx, j in enumerate(js):
                            # row-packed pair: head A on PE rows 0-63,
                            # head B on rows 64-127, concurrent
                            for half in range(2):
                                p0 = half * 64
                                nc.tensor.matmul(
                                    sc[:, (half * 3 + idx) * 512:
                                       (half * 3 + idx + 1) * 512],
                                    kt[p0:p0 + 64, j * 128:(j + 1) * 128],
                                    qt[p0:p0 + 64, :],
                                    start=True, stop=True)
                        pg = pp.tile([128, 6 * 512], F32R)
                        if blk == 3:
                            nc.scalar.activation(pg[:], sc[:], AF.Exp,
                                                 scale=SCALE)
                        else:
                            for half in range(2):
                                nc.scalar.activation(
                                    pg[:, half * 1536:half * 1536 + blk * 512],
                                    sc[:, half * 1536:half * 1536 + blk * 512],
                                    AF.Exp, scale=SCALE)
                        # software pipeline: PV of the previous block runs
                        # while ACT computes this block's exp
                        if prev is not None:
                            emit_pv(*prev)
                        prev = (pg, js)
                    emit_pv(*prev)
                    for half in range(2):
                        pv = pvs[half]
                        # copy [out|denom] rows out of PSUM immediately so the
                        # accumulator bank frees for the next iteration
                        uv = smalls.tile([65, 512], F32)
                        nc.vector.tensor_copy(uv[:], pv[0:65, :])
                        rec = smalls.tile([1, 512], F32)
                        nc.vector.reciprocal(rec[:], uv[64:65, :])
                        rb = smalls.tile([64, 512], F32)
                        nc.gpsimd.partition_broadcast(rb[:], rec[0:1, :])
                        o2 = smalls.tile([64, 512], F32)
                        nc.vector.tensor_mul(o2[:], uv[0:64, :], rb[:])
                        oslice = OT[half * 64:half * 64 + 64,
                                    hp * MYROWS + ich * 512:
                                    hp * MYROWS + (ich + 1) * 512]
                        nc.vector.tensor_scalar_add(
                            oslice, o2[:],
                            b_sb[half * 64:half * 64 + 64, 16 + hp:17 + hp])

        # ---------------- Phase C: out projection ----------------
        with ExitStack() as cctx:
            woT_pool = cctx.enter_context(tc.tile_pool(name="woT", bufs=2))
            yt_pool = cctx.enter_context(tc.tile_pool(name="yt", bufs=3))
            y_ps = cctx.enter_context(
                tc.tile_pool(name="y_ps", bufs=2, space="PSUM"))

            for et in range(2):
                woT = woT_pool.tile([128, CT * 512], F32R)  # [c_p, ct, 512 e]
                nc.sync.dma_start(woT[:], wo_in[et])
                for it in range(8):
                    y = y_ps.tile([128, 512], F32)
                    for ct in range(CT):
                        nc.tensor.matmul(
                            y[:],
                            OT[:, ct * MYROWS + it * 128: ct * MYROWS + (it + 1) * 128],
                            woT[:, ct * 512:(ct + 1) * 512],
                            start=(ct == 0), stop=(ct == CT - 1))
                    yt = yt_pool.tile([128, 512], F32)
                    nc.vector.tensor_add(yt[:], y[:], bo_bc[:, et * 512:(et + 1) * 512])
                    nc.sync.dma_start(
                        out[it * 128:(it + 1) * 128, et * 512:(et + 1) * 512], yt[:])

    nc.compile()
    return nc


_cache = {}


def _get_nc():
    if "nc" not in _cache:
        _cache["nc"] = build()
    return _cache["nc"]


def kernel(x_q, W_qkv, b_qkv, W_out, b_out):
    """Core c of 8 handles batch c//2, query rows [(c%2)*1024, +1024).

    The per-core x slice is ROLLED by the core's query-row offset so every
    core's own query rows sit at rows [0, MYROWS) of its slice. Attention is
    permutation-invariant over keys, so the rolled K/V ordering does not
    change the output.
    """
    x_q = np.ascontiguousarray(x_q, dtype=np.float32)
    W_qkv = np.ascontiguousarray(W_qkv, dtype=np.float32)
    b_qkv = np.ascontiguousarray(b_qkv, dtype=np.float32)
    W_out = np.ascontiguousarray(W_out, dtype=np.float32)
    b_out = np.ascontiguousarray(b_out, dtype=np.float32)

    nc = _get_nc()
    in_maps = build_in_maps(x_q, W_qkv, b_qkv, W_out, b_out)
    res = run_bass_kernel_spmd(nc, in_maps, list(range(NCORES)))
    out = np.empty((B, S, C), dtype=np.float32)
    for c in range(NCORES):
        b, half = c // 2, c % 2
        out[b, half * MYROWS:(half + 1) * MYROWS] = res.results[c]["out"]
    return out


def build_in_maps(x_q, W_qkv, b_qkv, W_out, b_out):
    x_q = np.ascontiguousarray(x_q, dtype=np.float32)
    W_qkv = np.asarray(W_qkv, dtype=np.float32)
    b_qkv = np.ascontiguousarray(b_qkv, dtype=np.float32)
    W_out = np.asarray(W_out, dtype=np.float32)
    b_out = np.ascontiguousarray(b_out, dtype=np.float32)
    # wqk[wt, p, ct*128+f] = W_qkv[wt*128+f, ct*128+p]
    w4 = W_qkv.reshape(24, 128, CT, 128)            # [wt, f, ct, p]
    wqk = np.ascontiguousarray(w4[:16].transpose(0, 3, 2, 1).reshape(
        16, 128, CT * 128))
    # wv[vch, p, ct*512+f] = W_qkv[2C+vch*512+f, ct*128+p]
    wv5 = W_qkv[2 * C:].reshape(2, 512, CT, 128)    # [vch, f, ct, p]
    wv = np.ascontiguousarray(wv5.transpose(0, 3, 2, 1).reshape(
        2, 128, CT * 512))
    # wo[et, p, ct*512+e] = W_out[et*512+e, ct*128+p]
    wo5 = W_out.reshape(2, 512, CT, 128)            # [et, e, ct, p]
    wo = np.ascontiguousarray(wo5.transpose(0, 3, 2, 1).reshape(
        2, 128, CT * 512))
    bq2d = np.ascontiguousarray(b_qkv.reshape(24, 128).T)
    in_maps = []
    for c in range(NCORES):
        b, half = c // 2, c % 2
        xb = x_q[b]
        if half:
            xb = np.roll(xb, -MYROWS, axis=0)
        in_maps.append({
            "xT": np.ascontiguousarray(xb.T),
            "wqk": wqk,
            "wv": wv,
            "wo": wo,
            "bq2d": bq2d,
            "b_out": b_out,
        })
    return in_maps


if __name__ == "__main__":
    # smoke test with random inputs
    rng = np.random.default_rng(0)
    x_q = rng.standard_normal((B, S, C), dtype=np.float32)
    s = 1.0 / np.sqrt(C)
    W_qkv = rng.uniform(-s, s, (3 * C, C)).astype(np.float32)
    b_qkv = rng.uniform(-s, s, 3 * C).astype(np.float32)
    W_out = rng.uniform(-s, s, (C, C)).astype(np.float32)
    b_out = rng.uniform(-s, s, C).astype(np.float32)
    got = kernel(x_q=x_q, W_qkv=W_qkv, b_qkv=b_qkv, W_out=W_out, b_out=b_out)
    print("smoke ok", got.shape, float(np.abs(got).max()))


# revision 30
# speedup vs baseline: 1.1202x; 1.0294x over previous
"""Multi-head attention (B=4, S=2048, C=1024, H=16) on 8 TRN2 NeuronCores.

Sharding: data-parallel over batch (4) x query-row split (2); core c handles
batch c//2, query rows [(c%2)*1024, +1024). The host rolls each core's x by
its query-row offset (attention is permutation-invariant over keys), passes
x^T and DMA-friendly re-layouts of the weights, and each core runs:

  A) QKV projection in float32r (full PE rate at N=512), Q^T/K^T produced
     feature-major, V natural; spilled to DRAM scratch.
  B) Per head-pair attention: transposed scores sc[j,i] = K_h^T(stationary)
     x Q_h^T(moving), both heads row-packed on PE array halves into one PSUM
     tile; one exp per 3-j-tile block (scale=1/8 folded into the ACT affine);
     P.V with a [V|1|0*63] 128-column stationary so the fp32r fast-weight-load
     stays on and the softmax denominator accumulates at out row 64; PV of
     block i-1 is emitted after exp of block i so it fills the PE while ACT
     runs. Normalization multiplies by the broadcast reciprocal denominator
     and folds the V bias in afterwards (sum_j softmax = 1).
  C) Out-projection with O^T as stationary so y lands in natural [row,
     channel] layout; bias added via a partition-broadcast tile.

No collectives; each core writes its own [1024, 1024] output slice.
"""

from contextlib import ExitStack

import numpy as np

import concourse.mybir as mybir
import concourse.tile as tile
from concourse import bacc
from concourse.bass_utils import run_bass_kernel_spmd
from concourse.masks import make_identity

F32 = mybir.dt.float32
F32R = mybir.dt.float32r
AF = mybir.ActivationFunctionType

B, S, C, H, DH = 4, 2048, 1024, 16, 64
NCORES = 8
SCALE = DH ** -0.5  # 0.125
CT = C // 128  # 8 channel tiles
ST = S // 128  # 16 seq tiles
MYROWS = S // 2  # 1024 query rows per core


def build():
    nc = bacc.Bacc("TRN2", target_bir_lowering=False, debug=False,
                   num_devices=NCORES)

    # host-prepared layouts (pure data movement on the host):
    #   xT[c, s] = x[s, c]
    #   wqk[wt, p, ct*128+f] = W_qkv[wt*128+f, ct*128+p]   (Q/K strips)
    #   wv[vch, p, ct*512+f] = W_qkv[2C+vch*512+f, ct*128+p]
    #   wo[et, p, ct*512+e] = W_out[et*512+e, ct*128+p]
    #   bq2d[p, wt] = b_qkv[wt*128+p]
    xT_in = nc.dram_tensor("xT", [C, S], F32R, kind="ExternalInput").ap()
    wqk_in = nc.dram_tensor("wqk", [16, 128, CT * 128], F32R,
                            kind="ExternalInput").ap()
    wv_in = nc.dram_tensor("wv", [2, 128, CT * 512], F32R,
                           kind="ExternalInput").ap()
    wo_in = nc.dram_tensor("wo", [2, 128, CT * 512], F32R,
                           kind="ExternalInput").ap()
    bq2d = nc.dram_tensor("bq2d", [128, 3 * C // 128], F32,
                          kind="ExternalInput").ap()
    b_out = nc.dram_tensor("b_out", [C], F32, kind="ExternalInput").ap()
    out = nc.dram_tensor("out", [MYROWS, C], F32, kind="ExternalOutput").ap()

    # DRAM scratch for the projected tensors (feature-major Q^T/K^T, natural V)
    qT_d = nc.dram_tensor("qT_d", [C, MYROWS], F32R).ap()
    kT_d = nc.dram_tensor("kT_d", [C, S], F32R).ap()
    v_d = nc.dram_tensor("v_d", [S, C], F32R).ap()

    with tile.TileContext(nc) as tc, ExitStack() as ctx:
        const = ctx.enter_context(tc.tile_pool(name="const", bufs=1))
        ident = const.tile([128, 128], F32)
        make_identity(nc, ident[:])  # used for the ones-column writes

        b_sb = const.tile([128, 3 * C // 128], F32)  # b_sb[p, wt] = b_qkv[wt*128+p]
        nc.sync.dma_start(b_sb[:], bq2d)
        bo_sb = const.tile([1, C], F32)
        nc.sync.dma_start(bo_sb[:], b_out[None, :])
        bo_bc = const.tile([128, C], F32)
        nc.gpsimd.partition_broadcast(bo_bc[:], bo_sb[0:1, :])

        persist = ctx.enter_context(tc.tile_pool(name="persist", bufs=1))
        xT = persist.tile([128, CT * S], F32R)  # xT[p, ct*S + s] = x[s, ct*128+p]

        # ---------------- Phase A: QKV projection ----------------
        with ExitStack() as actx:
            wstrip = actx.enter_context(tc.tile_pool(name="wstrip", bufs=3))
            vw_pool = actx.enter_context(tc.tile_pool(name="vw", bufs=2))
            stage = actx.enter_context(tc.tile_pool(name="stage", bufs=4))
            acc_ps = actx.enter_context(
                tc.tile_pool(name="acc_ps", bufs=3, space="PSUM"))

            # x^T resident: 8 c-tiles of [128, S]
            for ct in range(CT):
                nc.sync.dma_start(xT[:, ct * S:(ct + 1) * S],
                                  xT_in[ct * 128:(ct + 1) * 128, :])

            # V natural: rhs = W_v^T chunks [c_part, ct, 512 feats]
            for vch in range(2):
                vw = vw_pool.tile([128, CT * 512], F32R)
                nc.sync.dma_start(vw[:], wv_in[vch])
                for st in range(ST):
                    acc = acc_ps.tile([128, 512], F32)
                    for ct in range(CT):
                        nc.tensor.matmul(
                            acc[:],
                            xT[:, ct * S + st * 128: ct * S + (st + 1) * 128],
                            vw[:, ct * 512:(ct + 1) * 512],
                            start=(ct == 0), stop=(ct == CT - 1))
                    stg = stage.tile([128, 512], F32R)
                    nc.vector.tensor_copy(stg[:], acc[:])
                    nc.sync.dma_start(
                        v_d[st * 128:(st + 1) * 128,
                            vch * 512:(vch + 1) * 512], stg[:])

            # Q^T / K^T: per 128-feature strip, stream W^T column slices
            for wt in range(16):
                # ws[p, ct, f] = wT[ct*128+p, wt*128+f]
                ws = wstrip.tile([128, CT * 128], F32R)
                nc.sync.dma_start(ws[:], wqk_in[wt])
                if wt < 8:
                    nsch, dest, drow = 2, qT_d, wt
                else:
                    nsch, dest, drow = 4, kT_d, wt - 8
                for sch in range(nsch):
                    acc = acc_ps.tile([128, 512], F32)
                    for ct in range(CT):
                        nc.tensor.matmul(
                            acc[:],
                            ws[:, ct * 128:(ct + 1) * 128],
                            xT[:, ct * S + sch * 512: ct * S + sch * 512 + 512],
                            start=(ct == 0), stop=(ct == CT - 1))
                    stg = stage.tile([128, 512], F32R)
                    nc.vector.tensor_scalar_add(stg[:], acc[:],
                                                b_sb[:, wt:wt + 1])
                    nc.sync.dma_start(
                        dest[drow * 128:(drow + 1) * 128,
                             sch * 512:(sch + 1) * 512], stg[:])

        # ---------------- Phase B: attention ----------------
        OT = persist.tile([128, CT * MYROWS], F32R)  # OT[p, ct*1024 + i]
        with ExitStack() as bctx:
            kp = bctx.enter_context(tc.tile_pool(name="kp", bufs=2))
            vp = bctx.enter_context(tc.tile_pool(name="vp", bufs=2))
            qp = bctx.enter_context(tc.tile_pool(name="qp", bufs=2))
            pp = bctx.enter_context(tc.tile_pool(name="pp", bufs=2))
            smalls = bctx.enter_context(tc.tile_pool(name="smalls", bufs=3))
            sc_ps = bctx.enter_context(
                tc.tile_pool(name="sc_ps", bufs=1, space="PSUM"))
            pv_ps = bctx.enter_context(
                tc.tile_pool(name="pv_ps", bufs=1, space="PSUM"))

            for hp in range(H // 2):  # head pairs: A = rows 0-63, B = 64-127
                kt = kp.tile([128, S], F32R)
                nc.sync.dma_start(kt[:], kT_d[hp * 128:(hp + 1) * 128, :])
                # pair-wide V tile, padded to 128 stationary columns per
                # (j-tile, head): [V_A(64)|1|0*63|V_B(64)|1|0*63] per j-tile.
                # Full-width weights keep the fp32r fast-weight-load path;
                # the ones column accumulates the softmax denominator at out
                # row 64. One contiguous-chunk 1 MB DMA loads both heads.
                vt = vp.tile([128, ST * 256], F32R)
                vt4 = vt[:].rearrange("p (t g f) -> p t g f", g=2, f=128)
                nc.vector.tensor_scalar(
                    vt4[:, :, :, DH:DH + 1],
                    ident[:, 0:2 * ST].rearrange("p (t g) -> p t g", g=2),
                    0.0, 1.0, mybir.AluOpType.mult, mybir.AluOpType.add)
                for g in range(2):
                    nc.vector.tensor_scalar(
                        vt4[:, :, g:g + 1, DH + 1:128],
                        bo_bc[:, 0:ST * (127 - DH)].rearrange(
                            "p (t g f) -> p t g f", g=1, f=127 - DH),
                        0.0, 0.0, mybir.AluOpType.mult, mybir.AluOpType.mult)
                for g in range(2):
                    nc.sync.dma_start(
                        vt4[:, :, g:g + 1, 0:DH],
                        v_d.rearrange("(t p) (g f) -> p t g f", p=128, f=64)[
                            :, :, 2 * hp + g:2 * hp + g + 1, :])
                for ich in range(2):
                    qt = qp.tile([128, 512], F32R)
                    nc.sync.dma_start(
                        qt[:], qT_d[hp * 128:(hp + 1) * 128,
                                    ich * 512:(ich + 1) * 512])
                    pvs = [pv_ps.tile([128, 512], F32, tag=f"pv{half}",
                                      name=f"pv{half}")
                           for half in range(2)]
                    def emit_pv(pg_prev, js_prev):
                        for half in range(2):
                            for idx, j in enumerate(js_prev):
                                nc.tensor.matmul(
                                    pvs[half][:],
                                    vt[:, j * 256 + half * 128:
                                       j * 256 + half * 128 + 128],
                                    pg_prev[:, (half * 3 + idx) * 512:
                                            (half * 3 + idx + 1) * 512],
                                    start=(j == 0), stop=(j == 15))

                    jb = 0
                    prev = None
                    for blk in (3, 3, 3, 3, 2, 1, 1):
                        js = list(range(jb, jb + blk))
                        jb += blk
                        # one PSUM tile for both heads: A cols [0,1536),
                        # B cols [1536,3072) -> a single exp instruction
                        sc = sc_ps.tile([128, 6 * 512], F32)
                        for idx, j in enumerate(js):
                            # row-packed pair: head A on PE rows 0-63,
                            # head B on rows 64-127, concurrent
                            for half in range(2):
                                p0 = half * 64
                                nc.tensor.matmul(
                                    sc[:, (half * 3 + idx) * 512:
                                       (half * 3 + idx + 1) * 512],
                                    kt[p0:p0 + 64, j * 128:(j + 1) * 128],
                                    qt[p0:p0 + 64, :],
                                    start=True, stop=True)
                        pg = pp.tile([128, 6 * 512], F32R)
                        if blk == 3:
                            nc.scalar.activation(pg[:], sc[:], AF.Exp,
                                                 scale=SCALE)
                        else:
                            for half in range(2):
                                nc.scalar.activation(
                                    pg[:, half * 1536:half * 1536 + blk * 512],
                                    sc[:, half * 1536:half * 1536 + blk * 512],
                                    AF.Exp, scale=SCALE)
                        # software pipeline: PV of the previous block runs
                        # while ACT computes this block's exp
                        if prev is not None:
                            emit_pv(*prev)
                        prev = (pg, js)
                    emit_pv(*prev)
                    for half in range(2):
                        pv = pvs[half]
                        # copy [out|denom] rows out of PSUM immediately so the
                        # accumulator bank frees for the next iteration
                        uv = smalls.tile([65, 512], F32)
                        nc.vector.tensor_copy(uv[:], pv[0:65, :])
                        rec = smalls.tile([1, 512], F32)
                        nc.vector.reciprocal(rec[:], uv[64:65, :])
                        rb = smalls.tile([64, 512], F32)
                        nc.gpsimd.partition_broadcast(rb[:], rec[0:1, :])
                        o2 = smalls.tile([64, 512], F32)
                        nc.vector.tensor_mul(o2[:], uv[0:64, :], rb[:])
                        oslice = OT[half * 64:half * 64 + 64,
                                    hp * MYROWS + ich * 512:
                                    hp * MYROWS + (ich + 1) * 512]
                        nc.vector.tensor_scalar_add(
                            oslice, o2[:],
                            b_sb[half * 64:half * 64 + 64, 16 + hp:17 + hp])

        # ---------------- Phase C: out projection ----------------
        with ExitStack() as cctx:
            woT_pool = cctx.enter_context(tc.tile_pool(name="woT", bufs=2))
            yt_pool = cctx.enter_context(tc.tile_pool(name="yt", bufs=3))
            y_ps = cctx.enter_context(
                tc.tile_pool(name="y_ps", bufs=2, space="PSUM"))

            for et in range(2):
                woT = woT_pool.tile([128, CT * 512], F32R)  # [c_p, ct, 512 e]
                nc.sync.dma_start(woT[:], wo_in[et])
                for it in range(8):
                    y = y_ps.tile([128, 512], F32)
                    for ct in range(CT):
                        nc.tensor.matmul(
                            y[:],
                            OT[:, ct * MYROWS + it * 128: ct * MYROWS + (it + 1) * 128],
                            woT[:, ct * 512:(ct + 1) * 512],
                            start=(ct == 0), stop=(ct == CT - 1))
                    yt = yt_pool.tile([128, 512], F32)
                    nc.vector.tensor_add(yt[:], y[:], bo_bc[:, et * 512:(et + 1) * 512])
                    nc.sync.dma_start(
                        out[it * 128:(it + 1) * 128, et * 512:(et + 1) * 512], yt[:])

    nc.compile()
    return nc


_cache = {}


def _get_nc():
    if "nc" not in _cache:
        _cache["nc"] = build()
    return _cache["nc"]


def kernel(x_q, W_qkv, b_qkv, W_out, b_out):
    """Core c of 8 handles batch c//2, query rows [(c%2)*1024, +1024).

    The per-core x slice is ROLLED by the core's query-row offset so every
    core's own query rows sit at rows [0, MYROWS) of its slice. Attention is
    permutation-invariant over keys, so the rolled K/V ordering does not
    change the output.
    """
    x_q = np.ascontiguousarray(x_q, dtype=np.float32)
    W_qkv = np.ascontiguousarray(W_qkv, dtype=np.float32)
    b_qkv = np.ascontiguousarray(b_qkv, dtype=np.float32)
    W_out = np.ascontiguousarray(W_out, dtype=np.float32)
    b_out = np.ascontiguousarray(b_out, dtype=np.float32)

    nc = _get_nc()
    in_maps = build_in_maps(x_q, W_qkv, b_qkv, W_out, b_out)
    res = run_bass_kernel_spmd(nc, in_maps, list(range(NCORES)))
    out = np.empty((B, S, C), dtype=np.float32)
    for c in range(NCORES):
        b, half = c // 2, c % 2
        out[b, half * MYROWS:(half + 1) * MYROWS] = res.results[c]["out"]
    return out


def build_in_maps(x_q, W_qkv, b_qkv, W_out, b_out):
    x_q = np.ascontiguousarray(x_q, dtype=np.float32)
    W_qkv = np.asarray(W_qkv, dtype=np.float32)
    b_qkv = np.ascontiguousarray(b_qkv, dtype=np.float32)
    W_out = np.asarray(W_out, dtype=np.float32)
    b_out = np.ascontiguousarray(b_out, dtype=np.float32)
    # wqk[wt, p, ct*128+f] = W_qkv[wt*128+f, ct*128+p]
    w4 = W_qkv.reshape(24, 128, CT, 128)            # [wt, f, ct, p]
    wqk = np.ascontiguousarray(w4[:16].transpose(0, 3, 2, 1).reshape(
        16, 128, CT * 128))
    # wv[vch, p, ct*512+f] = W_qkv[2C+vch*512+f, ct*128+p]
    wv5 = W_qkv[2 * C:].reshape(2, 512, CT, 128)    # [vch, f, ct, p]
    wv = np.ascontiguousarray(wv5.transpose(0, 3, 2, 1).reshape(
        2, 128, CT * 512))
    # wo[et, p, ct*512+e] = W_out[et*512+e, ct*128+p]
    wo5 = W_out.reshape(2, 512, CT, 128)            # [et, e, ct, p]
    wo = np.ascontiguousarray(wo5.transpose(0, 3, 2, 1).reshape(
        2, 128, CT * 512))
    bq2d = np.ascontiguousarray(b_qkv.reshape(24, 128).T)
    in_maps = []
    for c in range(NCORES):
        b, half = c // 2, c % 2
        xb = x_q[b]
        if half:
            xb = np.roll(xb, -MYROWS, axis=0)
        in_maps.append({
            "xT": np.ascontiguousarray(xb.T),
            "wqk": wqk,
            "wv": wv,
            "wo": wo,
            "bq2d": bq2d,
            "b_out": b_out,
        })
    return in_maps


if __name__ == "__main__":
    # smoke test with random inputs
    rng = np.random.default_rng(0)
    x_q = rng.standard_normal((B, S, C), dtype=np.float32)
    s = 1.0 / np.sqrt(C)
    W_qkv = rng.uniform(-s, s, (3 * C, C)).astype(np.float32)
    b_qkv = rng.uniform(-s, s, 3 * C).astype(np.float32)
    W_out = rng.uniform(-s, s, (C, C)).astype(np.float32)
    b_out = rng.uniform(-s, s, C).astype(np.float32)
    got = kernel(x_q=x_q, W_qkv=W_qkv, b_qkv=b_qkv, W_out=W_out, b_out=b_out)
    print("smoke ok", got.shape, float(np.abs(got).max()))
